# revision 20
# baseline (speedup 1.0000x reference)
# Trainium2 Bass kernel for nn_AttentionPropagation (SuperGlue-style bidirectional
# attentional propagation): 6x (1x1conv+BN+ReLU) filters + QK attention with
# softmax over BOTH axes + two aggregations + output filters.
#
# Sharding: 16 (batch, head) units over 8 cores -> each core owns batch b=core//2
# and a contiguous 128-channel (2-head) slice of the filter outputs.
#
# v2 restructure vs baseline:
#  - chunked x1t/x2t input DMA so q/k matmuls (and the ACT exp stream) start early
#  - h2-major (m-half-major) streams: U-accumulator PSUM shrinks to [128,1024]
#    (2 banks) per half, freeing psS to 3 bufs (6 banks) -> deeper QK/exp pipe
#  - u-interleaved QK pairs (row groups 0-63/64-127) can overlap on the PE
#  - rows norm chains emitted right after the E-stream (overlap F-stream)
#  - U0 eviction fused with rowsum normalization (tensor_mul from PSUM)
#  - collective split: CC1 AllGathers unnormalized add1 halves at E-stream end
#    (hidden under the F-stream); CC2b (colinv flats, f32) + CC2a (normalized
#    add0 halves) at F end; all collective outputs addr_space="Shared"
#  - phase 5 (f4+f5) on every core from absolute-layout gathered tiles; colinv
#    partition-broadcast via K=1 matmuls into PSUM (streams done, PSUM free)

import numpy as np

B, N, M, C = 4, 2048, 2048, 256
H, Dh = 4, 64
EPS = 1e-5
NCORES = 8

_CACHE = {}


def _build_program():
    from contextlib import ExitStack

    import concourse.bass as bass
    import concourse.tile as tile
    from concourse import bacc, mybir
    from concourse.bass import ts

    f32 = mybir.dt.float32
    bf16 = mybir.dt.bfloat16
    AF = mybir.ActivationFunctionType
    ALU = mybir.AluOpType

    nc = bacc.Bacc(
        "TRN2",
        target_bir_lowering=False,
        debug=False,
        enable_asserts=False,
        num_devices=NCORES,
    )

    # ---- DRAM I/O ----
    x1t_d = nc.dram_tensor("x1t", [C, N], bf16, kind="ExternalInput").ap()
    x2t_d = nc.dram_tensor("x2t", [C, M], bf16, kind="ExternalInput").ap()
    wq_d = nc.dram_tensor("wq", [C, 128], bf16, kind="ExternalInput").ap()
    wk_d = nc.dram_tensor("wk", [C, 128], bf16, kind="ExternalInput").ap()
    wv0_d = nc.dram_tensor("wv0", [C, 128], bf16, kind="ExternalInput").ap()
    wv1_d = nc.dram_tensor("wv1", [C, 128], bf16, kind="ExternalInput").ap()
    bq_d = nc.dram_tensor("bq", [128, 1], f32, kind="ExternalInput").ap()
    bk_d = nc.dram_tensor("bk", [128, 1], f32, kind="ExternalInput").ap()
    bv0_d = nc.dram_tensor("bv0", [1, 128], bf16, kind="ExternalInput").ap()
    bv1_d = nc.dram_tensor("bv1", [1, 128], bf16, kind="ExternalInput").ap()
    w4t_d = nc.dram_tensor("w4t", [C, C], bf16, kind="ExternalInput").ap()
    w5t_d = nc.dram_tensor("w5t", [C, C], bf16, kind="ExternalInput").ap()
    b4_d = nc.dram_tensor("b4", [1, C], bf16, kind="ExternalInput").ap()
    b5_d = nc.dram_tensor("b5", [1, C], bf16, kind="ExternalInput").ap()
    ones_d = nc.dram_tensor("ones", [1, 128], bf16, kind="ExternalInput").ap()
    onesf_d = nc.dram_tensor("onesf", [1, 128], f32, kind="ExternalInput").ap()
    out0_d = nc.dram_tensor("out0t", [N, C], f32, kind="ExternalOutput").ap()
    out1_d = nc.dram_tensor("out1t", [M, C], f32, kind="ExternalOutput").ap()
    cc1_in = nc.dram_tensor("cc1_in", [128, M], bf16, kind="Internal").ap()
    cc1_out = nc.dram_tensor(
        "cc1_out", [256, M], bf16, kind="Internal"
    ).ap()
    cc2a_in = nc.dram_tensor("cc2a_in", [128, N], bf16, kind="Internal").ap()
    cc2a_out = nc.dram_tensor(
        "cc2a_out", [256, N], bf16, kind="Internal"
    ).ap()
    cc2b_in = nc.dram_tensor("cc2b_in", [2, M], f32, kind="Internal").ap()
    cc2b_out = nc.dram_tensor(
        "cc2b_out", [4, M], f32, kind="Internal"
    ).ap()
    sc_d = nc.dram_tensor("sc", [4, 128, 16], f32, kind="Internal").ap()

    NB = N // 128  # 16 n-blocks
    MB = M // 128  # 16 m-blocks
    PAIRS = [[0, 1], [2, 3], [4, 5], [6, 7]]

    with tile.TileContext(nc) as tc, ExitStack() as ctx:
        const = ctx.enter_context(tc.tile_pool(name="const", bufs=1))
        # x1t/x2t (phase 1) and af1/af0 (tail) share two slots via one tag
        xpool = ctx.enter_context(tc.tile_pool(name="xp", bufs=2))
        qkp = ctx.enter_context(tc.tile_pool(name="qkp", bufs=1))
        vp = ctx.enter_context(tc.tile_pool(name="vp", bufs=1))
        accp = ctx.enter_context(tc.tile_pool(name="accp", bufs=1))
        addp = ctx.enter_context(tc.tile_pool(name="addp", bufs=1))
        bcp = ctx.enter_context(tc.tile_pool(name="bcp", bufs=1))
        stream = ctx.enter_context(tc.tile_pool(name="stream", bufs=14))
        opool = ctx.enter_context(tc.tile_pool(name="opool", bufs=3))
        # PSUM: psS = 3 bufs x [128,1024] (2 banks each) ; psU = 1 x [128,1024]
        psS = ctx.enter_context(tc.tile_pool(name="psS", bufs=3, space="PSUM"))
        psU = ctx.enter_context(tc.tile_pool(name="psU", bufs=1, space="PSUM"))

        # ---- constants ----
        wq_sb = const.tile([128, 2, 128], bf16, tag="wq")
        wk_sb = const.tile([128, 2, 128], bf16, tag="wk")
        wv0_sb = const.tile([128, 2, 128], bf16, tag="wv0")
        wv1_sb = const.tile([128, 2, 128], bf16, tag="wv1")
        w4t_sb = const.tile([128, 2, 256], bf16, tag="w4t")
        w5t_sb = const.tile([128, 2, 256], bf16, tag="w5t")
        for dst, src in ((wq_sb, wq_d), (wk_sb, wk_d), (wv0_sb, wv0_d), (wv1_sb, wv1_d)):
            nc.sync.dma_start(dst[:], src.rearrange("(a p) d -> p a d", p=128))
        for dst, src in ((w4t_sb, w4t_d), (w5t_sb, w5t_d)):
            nc.sync.dma_start(dst[:], src.rearrange("(a p) d -> p a d", p=128))
        bq_sb = const.tile([128, 1], f32, tag="bq")
        bk_sb = const.tile([128, 1], f32, tag="bk")
        bv0_sb = const.tile([1, 128], bf16, tag="bv0")
        bv1_sb = const.tile([1, 128], bf16, tag="bv1")
        b4_sb = const.tile([1, 256], bf16, tag="b4")
        b5_sb = const.tile([1, 256], bf16, tag="b5")
        for dst, src in (
            (bq_sb, bq_d), (bk_sb, bk_d), (bv0_sb, bv0_d),
            (bv1_sb, bv1_d), (b4_sb, b4_d), (b5_sb, b5_d),
        ):
            nc.sync.dma_start(dst[:], src)
        ones_t = const.tile([1, 128], bf16, tag="ones")
        nc.sync.dma_start(ones_t[:], ones_d)
        onesf_t = const.tile([1, 128], f32, tag="onesf")
        nc.sync.dma_start(onesf_t[:], onesf_d)

        # ---- inputs, chunked so q/k can start on chunk 0 early ----
        x1t_sb = xpool.tile([128, 2, N], bf16, tag="xa")
        x2t_sb = xpool.tile([128, 2, M], bf16, tag="xa")
        x1r = x1t_d.rearrange("(a p) n -> p a n", p=128)
        x2r = x2t_d.rearrange("(a p) n -> p a n", p=128)
        for j in range(4):
            nc.sync.dma_start(x1t_sb[:, :, ts(j, 512)], x1r[:, :, ts(j, 512)])
            nc.sync.dma_start(x2t_sb[:, :, ts(j, 512)], x2r[:, :, ts(j, 512)])

        # ---- phase 1a: q,k filters (chunk-interleaved) ----
        q_sb = qkp.tile([128, N], bf16, tag="q")
        k_sb = qkp.tile([128, M], bf16, tag="k")
        for j in range(4):  # 512-wide chunks
            for dst, xt, w, bias in (
                (q_sb, x1t_sb, wq_sb, bq_sb), (k_sb, x2t_sb, wk_sb, bk_sb)
            ):
                ps = psS.tile([128, 1024], f32, tag="s")
                p5 = ps[:, 0:512]
                nc.tensor.matmul(
                    p5, w[:, 0], xt[:, 0, ts(j, 512)], start=True, stop=False
                )
                nc.tensor.matmul(
                    p5, w[:, 1], xt[:, 1, ts(j, 512)], start=False, stop=True
                )
                nc.vector.tensor_scalar(
                    dst[:, ts(j, 512)], p5, bias[:], 0.0, op0=ALU.add, op1=ALU.max
                )

        # ---- v tiles: [m-in-block, mb*128+d] layout (lhsT for U matmuls) ----
        v0t_sb = vp.tile([128, MB * 128], bf16, tag="v0t")
        v1t_sb = vp.tile([128, NB * 128], bf16, tag="v1t")

        def emit_vtile(dst, xt, w, brow, mb):
            ps = psS.tile([128, 1024], f32, tag="s")
            p1 = ps[:, 0:128]
            nc.tensor.matmul(p1, xt[:, 0, ts(mb, 128)], w[:, 0], start=True, stop=False)
            nc.tensor.matmul(p1, xt[:, 1, ts(mb, 128)], w[:, 1], start=False, stop=False)
            nc.tensor.matmul(p1, ones_t[:, 0:128], brow[:], start=False, stop=True)
            nc.vector.tensor_scalar_max(dst[:, ts(mb, 128)], p1, 0.0)

        def _chunk_thunks(thunks, per=3):
            return [thunks[i : i + per] for i in range(0, len(thunks), per)]

        # accumulator scratch for softmax sums: col index = h2*16 + blk
        rows_acc = [
            accp.tile([128, 32], f32, tag=f"ra{u}", name=f"rows_acc{u}")
            for u in range(2)
        ]
        cols_acc = [
            accp.tile([128, 32], f32, tag=f"ca{u}", name=f"cols_acc{u}")
            for u in range(2)
        ]

        # ---- attention stream (used for both E and F passes) ----
        # nb-major with u0/u1 interleaved: the two QK matmuls use row groups
        # 0-63 / 64-127 and the two U matmuls use col groups (tile_position
        # (0,0)/(0,64)), so adjacent issue lets each pair run CONCURRENTLY on
        # the PE array -- the cold-clock (HAM K=4/8) PE then still fits under
        # the ACT exp cadence and the stream stays ACT-bound.
        # U(u1, nb==0) uses start=False: u0's start=True already cleared the
        # bank's has_written bits, so u1's first write lands as overwrite.
        def attn_stream(qs_sb, ks_sb, vt_sb, acc, add_sb, fuse_rbc, vtile_work):
            """qs: [128,2048] queries (n side), ks: keys (m side), vt: lhsT tiles,
            acc: per-u [128,32] accum tiles, add_sb: [128,2048] bf16 eviction dst,
            fuse_rbc: None (plain copy evict) or [rbc_u] for fused normalize,
            vtile_work: thunks emitting v-tile matmuls, injected into early
            PE slack (before the lagged U matmuls start)."""
            for h2 in range(2):
                Ups = psU.tile([128, 1024], f32, tag="u")

                def emit_u(u, nb, et):
                    for j in range(2):
                        nc.tensor.matmul(
                            Ups[64 * u : 64 * u + 64, ts(j, 512)],
                            vt_sb[:, nb * 128 + 64 * u : nb * 128 + 64 * u + 64],
                            et[:, ts(j, 512)],
                            start=(nb == 0),
                            stop=(nb == NB - 1),
                            tile_position=(0, 64 * u),
                        )

                for u in range(2):
                    pend = []
                    for nb in range(NB):
                        qs = qs_sb[64 * u : 64 * u + 64, ts(nb, 128)]
                        ps = psS.tile([128, 1024], f32, tag="s")
                        for j in range(2):
                            nc.tensor.matmul(
                                ps[:, ts(j, 512)], qs,
                                ks_sb[64 * u : 64 * u + 64,
                                      h2 * 1024 + 512 * j : h2 * 1024 + 512 * (j + 1)],
                                start=True, stop=True,
                            )
                        et = stream.tile([128, 1024], bf16, tag="st")
                        nc.scalar.activation(
                            et[:], ps[:], AF.Exp, scale=0.125,
                            accum_out=acc[u][:, h2 * 16 + nb : h2 * 16 + nb + 1],
                        )
                        pend.append((u, nb, et))
                        if len(pend) > 3:
                            emit_u(*pend.pop(0))
                        if vtile_work and nb == 1:
                            for thunk in [t for ch in vtile_work for t in ch]:
                                thunk()
                            vtile_work = None
                    for item in pend:
                        emit_u(*item)
                # evict this m-half (optionally fused with normalizer mul)
                if fuse_rbc is None:
                    nc.vector.tensor_copy(add_sb[:, ts(h2, 1024)], Ups[:])
                else:
                    for u in range(2):
                        nc.vector.tensor_mul(
                            add_sb[64 * u : 64 * u + 64, ts(h2, 1024)],
                            Ups[64 * u : 64 * u + 64, :],
                            fuse_rbc[u][64 * u : 64 * u + 64, ts(h2, 1024)],
                        )

        # ---- phase 2: E-stream (QK -> exp -> U1, rowsum accums) ----
        add1_sb = addp.tile([128, M], bf16, tag="a1")
        v1work = _chunk_thunks(
            [(lambda mb=mb: emit_vtile(v1t_sb, x1t_sb, wv1_sb, bv1_sb, mb))
             for mb in range(NB)]
        )
        attn_stream(q_sb, k_sb, v1t_sb, rows_acc, add1_sb, None, v1work)

        # CC1: exchange unnormalized add1 halves now; F-stream hides it
        nc.gpsimd.dma_start(cc1_in[:, :], add1_sb[:])
        nc.gpsimd.collective_compute(
            "AllGather", ALU.bypass, replica_groups=PAIRS,
            ins=[cc1_in], outs=[cc1_out],
        )
        af1 = xpool.tile([128, 2, M], bf16, tag="xa")
        nc.gpsimd.dma_start(af1[:, 0, :], cc1_out[0:128, :])
        nc.gpsimd.dma_start(af1[:, 1, :], cc1_out[128:256, :])

        # rows norm chains (rowsums ready; runs under the F-stream)
        rbc0 = []
        for u in range(2):
            s16 = accp.tile([128, 16], f32, tag=f"s16r{u}", name=f"s16r{u}")
            nc.vector.tensor_add(s16[:], rows_acc[u][:, 0:16], rows_acc[u][:, 16:32])
            rec16 = accp.tile([128, 16], f32, tag=f"r16r{u}", name=f"r16r{u}")
            nc.vector.reciprocal(rec16[:], s16[:])
            scs = sc_d[u]
            nc.gpsimd.dma_start(scs, rec16[:])
            flat = accp.tile([1, 2048], f32, tag=f"flr{u}", name=f"flr{u}")
            nc.gpsimd.dma_start(flat[:], scs.rearrange("p i -> i p"))
            rbc = bcp.tile([128, 2048], f32, tag=f"bcr{u}", name=f"bcr{u}")
            nc.gpsimd.partition_broadcast(rbc[:], flat[:])
            rbc0.append(rbc)

        # ---- phase 3: F-stream (QKT -> exp -> U0, colsum accums) ----
        add0_sb = addp.tile([128, N], bf16, tag="a0")
        v0work = _chunk_thunks(
            [(lambda mb=mb: emit_vtile(v0t_sb, x2t_sb, wv0_sb, bv0_sb, mb))
             for mb in range(MB)]
        )
        attn_stream(k_sb, q_sb, v0t_sb, cols_acc, add0_sb, rbc0, v0work)

        # ---- cols chains -> colinv flats in DRAM -> CC2b; add0 -> CC2a ----
        for u in range(2):
            s16 = accp.tile([128, 16], f32, tag=f"s16c{u}", name=f"s16c{u}")
            nc.vector.tensor_add(s16[:], cols_acc[u][:, 0:16], cols_acc[u][:, 16:32])
            rec16 = accp.tile([128, 16], f32, tag=f"r16c{u}", name=f"r16c{u}")
            nc.vector.reciprocal(rec16[:], s16[:])
            scs = sc_d[2 + u]
            nc.gpsimd.dma_start(scs, rec16[:])
            flatc = accp.tile([1, 2048], f32, tag=f"flc{u}", name=f"flc{u}")
            nc.gpsimd.dma_start(flatc[:], scs.rearrange("p i -> i p"))
            nc.gpsimd.dma_start(cc2b_in[u : u + 1, :], flatc[:])
        nc.gpsimd.collective_compute(
            "AllGather", ALU.bypass, replica_groups=PAIRS,
            ins=[cc2b_in], outs=[cc2b_out],
        )
        nc.gpsimd.dma_start(cc2a_in[:, :], add0_sb[:])
        nc.gpsimd.collective_compute(
            "AllGather", ALU.bypass, replica_groups=PAIRS,
            ins=[cc2a_in], outs=[cc2a_out],
        )

        # ---- tail: broadcast colinv, normalize af1, output filters ----
        flats = []
        for h in range(4):
            fl = accp.tile([1, 2048], f32, tag=f"flg{h}", name=f"flg{h}")
            nc.gpsimd.dma_start(fl[:], cc2b_out[h : h + 1, :])
            flats.append(fl)
        af0 = xpool.tile([128, 2, N], bf16, tag="xa")
        nc.gpsimd.dma_start(af0[:, 0, :], cc2a_out[0:128, :])
        nc.gpsimd.dma_start(af0[:, 1, :], cc2a_out[128:256, :])

        af1n = addp.tile([128, 2, M], bf16, tag="a1n")
        for s in range(2):
            for u in range(2):
                rbc1 = bcp.tile(
                    [128, 2048], f32, tag=f"bcc{2 * s + u}", name=f"bcc{2 * s + u}"
                )
                nc.gpsimd.partition_broadcast(rbc1[:], flats[2 * s + u][:])
                nc.vector.tensor_mul(
                    af1n[64 * u : 64 * u + 64, s, :],
                    af1[64 * u : 64 * u + 64, s, :],
                    rbc1[64 * u : 64 * u + 64, :],
                )

        # ---- phase 5: output filters, out-transposed ----
        for out_d, af, wt, brow in (
            (out0_d, af0, w4t_sb, b4_sb),
            (out1_d, af1n, w5t_sb, b5_sb),
        ):
            for nb in range(NB):
                ps = psS.tile([128, 1024], f32, tag="s")
                p2 = ps[:, 0:256]
                nc.tensor.matmul(
                    p2, af[:, 0, ts(nb, 128)], wt[:, 0], start=True, stop=False
                )
                nc.tensor.matmul(
                    p2, af[:, 1, ts(nb, 128)], wt[:, 1], start=False, stop=False
                )
                nc.tensor.matmul(
                    p2, ones_t[:, 0:128], brow[:], start=False, stop=True
                )
                ot = opool.tile([128, 256], f32, tag="ot")
                nc.vector.tensor_scalar_max(ot[:], p2, 0.0)
                nc.sync.dma_start(out_d[ts(nb, 128), :], ot[:])

    nc.compile()
    return nc


def _prep_core_inputs(inputs):
    """Fold BN into weights, build per-core input maps."""
    x1 = np.ascontiguousarray(inputs["x1"], dtype=np.float32)
    x2 = np.ascontiguousarray(inputs["x2"], dtype=np.float32)
    Ws = np.asarray(inputs["Ws"], dtype=np.float32)
    bs = np.asarray(inputs["bs"], dtype=np.float32)
    g = np.asarray(inputs["gammas"], dtype=np.float32)
    be = np.asarray(inputs["betas"], dtype=np.float32)
    mn = np.asarray(inputs["means"], dtype=np.float32)
    vr = np.asarray(inputs["vars_"], dtype=np.float32)

    s = g / np.sqrt(vr + EPS)  # [6, C]
    Wf = Ws * s[:, :, None]  # rows scaled
    bf = s * (bs - mn) + be

    import ml_dtypes

    bfl = ml_dtypes.bfloat16

    WfT = np.ascontiguousarray(np.swapaxes(Wf, 1, 2)).astype(bfl)  # [6, C, C]
    x1t = np.ascontiguousarray(np.swapaxes(x1, 1, 2)).astype(bfl)  # [B, C, N]
    x2t = np.ascontiguousarray(np.swapaxes(x2, 1, 2)).astype(bfl)
    bfb = bf.astype(bfl)

    in_maps = []
    for core in range(NCORES):
        b, par = core // 2, core % 2
        sl = slice(par * 128, par * 128 + 128)
        in_maps.append(
            {
                "x1t": x1t[b],
                "x2t": x2t[b],
                "wq": np.ascontiguousarray(WfT[0][:, sl]),
                "wk": np.ascontiguousarray(WfT[1][:, sl]),
                "wv0": np.ascontiguousarray(WfT[2][:, sl]),
                "wv1": np.ascontiguousarray(WfT[3][:, sl]),
                "bq": np.ascontiguousarray(bf[0][sl]).reshape(128, 1),
                "bk": np.ascontiguousarray(bf[1][sl]).reshape(128, 1),
                "bv0": np.ascontiguousarray(bfb[2][sl]).reshape(1, 128),
                "bv1": np.ascontiguousarray(bfb[3][sl]).reshape(1, 128),
                "w4t": WfT[4],
                "w5t": WfT[5],
                "b4": bfb[4].reshape(1, C),
                "b5": bfb[5].reshape(1, C),
                "ones": np.ones((1, 128), bfl),
                "onesf": np.ones((1, 128), np.float32),
            }
        )
    return in_maps


def kernel(**inputs):
    from concourse import bass_utils

    if "nc" not in _CACHE:
        _CACHE["nc"] = _build_program()
    nc = _CACHE["nc"]

    in_maps = _prep_core_inputs(inputs)
    res = bass_utils.run_bass_kernel_spmd(
        nc, in_maps, core_ids=list(range(NCORES))
    )
    results = res.results
    out0 = np.stack([results[2 * b]["out0t"] for b in range(B)])
    out1 = np.stack([results[2 * b]["out1t"] for b in range(B)])
    return out0, out1
